# revision 15
# baseline (speedup 1.0000x reference)
"""MoE layer (8 experts, top-2) on 8 Trainium2 NeuronCores.

Strategy: expert parallelism with 2-segment load balancing. The router
(x @ gate_w.T -> top-2 -> softmax) is computed on host in fp32 (0.03% of
total FLOPs). Tokens are sharded BY EXPERT, but instead of padding every
core to the max expert load (2084 for this seed), each core runs TWO
fixed-length segments (L1, L2) with independent weight sets; expert token
lists are split across segments on different cores so the per-core
capacity drops to the optimal 2-segment covering (2068 = L1 1042 +
L2 1026 here, vs the 2048 ideal). Weight DMA traffic is unchanged (the
full per-expert weight set was already streamed once per token group).

Each core computes the dense expert MLP
    y = gelu(x @ w1[e].T + b1[e]) @ w2[e].T
in bf16 (fp32 PSUM accumulation). The combine (scatter-add weighted by
the top-2 softmax coefficients, with b2[e] folded in per expert) happens
on host as the unshard step.

Device kernel layout (per core, SPMD identical program):
  xT  [D_IN, C]  bf16   tokens, transposed (partition dim = contraction)
  w1p [2, 64, 128, 2048]   per-segment w1 tiles:
                        w1p[g, h0, p, kt*128+h] = w1[e_g][h0*128+h, kt*128+p]
  w2p [2, 8, 4, 128, 4096] per-segment w2 [hid128, dout128] tiles, grouped
                        by (hid block, dout quarter) so matmul-2 keeps w2
                        stationary and streams token columns
  b1c [128, 128]        b1 per hid-tile column, both segments packed
  y   [D_OUT, C] f32    expert output, transposed (excl. b2/routing coef)

The tensor engine runs at the bf16 issue-rate floor in steady state
(measured 162.5 ns per FD-384 matmul = N/2.4GHz + 2.5ns dispatch), so
the remaining overheads addressed here: HAM cold-clock ramp (warmup
matmuls on a zeroed tile during the DMA prologue), prologue DMA ordering
(first matmul waits only on a 32KB weight chunk + 128KB token chunk),
and capacity padding (segment covering above).
"""

import numpy as np
import ml_dtypes

TOP_K = 2
NUM_EXPERTS = 8
D_IN, D_HID, D_OUT = 2048, 8192, 2048

P = 128
KT = D_IN // P     # 16 contraction tiles
NBLK = 8           # hid blocks of 1024
HPB = 8            # hid 128-tiles per block
NDC = 4            # dout quarters; D_OUT = 4 * 512
DOUTW = 512
_BF16 = ml_dtypes.bfloat16

_nc_cache: dict[tuple, object] = {}

LAST_EXEC_TIME_NS = None
LAST_RESULTS = None


def _widths_for(tg: int) -> list[int]:
    """Split a group into the minimum number (ceil(tg/512)) of matmul
    moving widths, 128-aligned starts except the last, every chunk >= 64
    (the tensor engine has a ~60-cycle issue floor)."""
    n = -(-tg // 512)
    if n <= 1:
        return [tg]
    ws = [512] * (n - 2)
    rem = tg - 512 * (n - 2)
    if rem - 512 >= 64:
        ws += [512, rem - 512]
    else:
        ws += [384, rem - 384]
    assert all(w >= 64 for w in ws) and sum(ws) == tg
    return ws


def _plan_segments(counts):
    """Optimal 2-segment covering: per-core capacity C = L1 + L2 with the
    8 experts' token lists packed into 8 L1-segments + 8 L2-segments
    (each segment single-expert). Returns (L1, L2) and per-core
    [(expert, start, len), (expert, start, len)]."""
    counts = np.asarray(counts)
    order = np.argsort(-counts, kind="stable")
    cs = counts[order]
    best = None
    for n1 in range(0, 5):
        n3, n2 = n1, 8 - 2 * n1
        if n2 < 0:
            continue
        L1 = int(-(-cs[:n1].max() // 2)) if n1 else 0
        req_mid = int(cs[n1:n1 + n2].max()) if n2 else 0
        L2 = int(-(-cs[n1 + n2:].max() // 2)) if n3 else 0
        if n1 == 0:
            # all experts whole on one core: L1+L2 >= max count
            C = max(req_mid, 2 * 512)
            L1 = C - C // 2
        else:
            C = max(L1 + L2, req_mid, L1 + 512)
        if best is None or C < best[0]:
            best = (C, n1, n2, L1, L2)
    C, n1, n2, L1min, L2min = best
    # slide the split within [L1min, C - L2min] to minimize the total
    # matmul width-chunk count (2.5ns dispatch per chunk per kt pass)
    L1min, L2min = max(L1min, 512), max(L2min, 512)
    L1 = min(
        range(L1min, C - L2min + 1),
        key=lambda l1: (len(_widths_for(l1)) + len(_widths_for(C - l1)),
                        -min(_widths_for(l1) + _widths_for(C - l1))),
    )
    L2 = C - L1
    segs1, segs2 = [], []
    for i, e in enumerate(order):
        c = int(counts[e])
        if i < n1:                       # two L1 segments
            segs1.append((e, 0, min(L1, c)))
            segs1.append((e, L1, max(0, c - L1)))
        elif i < n1 + n2:                # L1 + L2
            segs1.append((e, 0, min(L1, c)))
            segs2.append((e, L1, max(0, c - L1)))
        else:                            # two L2 segments
            segs2.append((e, 0, min(L2, c)))
            segs2.append((e, L2, max(0, c - L2)))
    assert len(segs1) == len(segs2) == 8
    for e, s, l in segs1:
        assert l <= L1
    for e, s, l in segs2:
        assert l <= L2
    cores = [(segs1[i], segs2[i]) for i in range(8)]
    return (L1, L2), cores


def _build_bass(groups: tuple[int, ...]):
    from concourse import bacc
    import concourse.mybir as mybir
    import concourse.tile as tile

    bf16 = mybir.dt.bfloat16
    f32 = mybir.dt.float32
    C = sum(groups)
    tgmax = max(groups)
    ngrp = len(groups)

    nc = bacc.Bacc("TRN2", target_bir_lowering=False, debug=False,
                   num_devices=NUM_EXPERTS)
    xT = nc.declare_dram_parameter("xT", [D_IN, C], bf16, isOutput=False)
    w1p = nc.declare_dram_parameter("w1p", [ngrp, D_HID // P, P, D_IN], bf16,
                                    isOutput=False)
    w2p = nc.declare_dram_parameter("w2p", [ngrp, NBLK, NDC, P, HPB * NDC * P],
                                    bf16, isOutput=False)
    b1c = nc.declare_dram_parameter("b1c", [P, ngrp * (D_HID // P)], f32,
                                    isOutput=False)
    y = nc.declare_dram_parameter("y", [D_OUT, C], f32, isOutput=True)

    gelu = mybir.ActivationFunctionType.Gelu

    with tile.TileContext(nc) as tc:
        with (
            tc.tile_pool(name="consts", bufs=1) as cpool,
            tc.tile_pool(name="xpool", bufs=1) as xpool,
            tc.tile_pool(name="ypool", bufs=1) as ypool,
            tc.tile_pool(name="hpool", bufs=2) as hpool,
            tc.tile_pool(name="w1pool", bufs=3) as w1pool,
            tc.tile_pool(name="w2pool", bufs=3) as w2pool,
            tc.tile_pool(name="phpool", bufs=4, space="PSUM") as phpool,
            tc.tile_pool(name="pypool", bufs=4, space="PSUM") as pypool,
        ):
            # prologue: the first accumulation (kt 0) waits only on a 32KB
            # weight chunk and a 96KB token chunk. The startup burst
            # (w1 tile + 16 token tiles, ~4.8MB) is split across the two
            # hardware DGE queues (SP + Activation) so it keeps pace with
            # the first DMA-paced hid-tile pass; those matmuls double as
            # the HAM clock-gate warmup.
            w0 = _widths_for(groups[0])[0]
            xs0 = xpool.tile([P, tgmax], bf16, tag="x0", name="xs0")
            nc.sync.dma_start(xs0[:, :w0], xT[0:P, 0:w0])
            w1a = cpool.tile([P, P], bf16, tag="w1a")
            nc.scalar.dma_start(w1a[:], w1p[0, 0, :, :P])
            w1b = w1pool.tile([P, D_IN], bf16, tag="w1")
            nc.scalar.dma_start(w1b[:, P:], w1p[0, 0, :, P:])
            if w0 < groups[0]:
                nc.sync.dma_start(xs0[:, w0:groups[0]],
                                  xT[0:P, w0:groups[0]])
            b1t = cpool.tile([P, ngrp * (D_HID // P)], f32)

            g0 = 0
            for g, tg in enumerate(groups):
                widths = _widths_for(tg)
                xs = [xs0 if (g == 0 and kt == 0) else
                      xpool.tile([P, tgmax], bf16, tag=f"x{kt}",
                                 name=f"xs{kt}")
                      for kt in range(KT)]
                if g == 0:
                    nc.sync.dma_start(
                        xs[1][:, :tg], xT[P:2 * P, g0:g0 + tg])
                    nc.scalar.dma_start(b1t[:], b1c[:])
                    for kt in range(2, KT):
                        eng = nc.sync if kt % 2 == 0 else nc.scalar
                        eng.dma_start(
                            xs[kt][:, :tg],
                            xT[kt * P:(kt + 1) * P, g0:g0 + tg])
                else:
                    for kt in range(KT):
                        nc.sync.dma_start(
                            xs[kt][:, :tg],
                            xT[kt * P:(kt + 1) * P, g0:g0 + tg])
                ys = [ypool.tile([P, tgmax], f32, tag=f"y{t}", name=f"ys{t}")
                      for t in range(D_OUT // P)]
                for b in range(NBLK):
                    hs = [hpool.tile([P, tgmax], bf16, tag=f"h{i}",
                                     name=f"hs{i}")
                          for i in range(HPB)]
                    # ---- matmul 1: h[hid, tok] = w1 @ x, gelu ----
                    hb_lo = 0
                    if g == 0 and b == 0:
                        # The startup x/w burst (~5MB) is chip-HBM-bound
                        # (~13us with all 8 cores loading at once), so run
                        # the first TWO hid-tiles fused kt-major: ~14us of
                        # matmul paced against the arriving xs tiles keeps
                        # the PE gapless (doubles as HAM clock warmup).
                        # 6 concurrent PSUM accumulation groups: 3 from
                        # phpool + 3 borrowed from pypool (mm2 starts
                        # long after these drain).
                        w1t1 = w1pool.tile([P, D_IN], bf16, tag="w1",
                                           name="w1t1")
                        nc.scalar.dma_start(w1t1[:], w1p[0, 1])
                        phsA = [phpool.tile([P, 512], mybir.dt.float32,
                                            tag="ph", name=f"phA{wi}")
                                for wi in range(len(widths))]
                        phsB = [pypool.tile([P, DOUTW], mybir.dt.float32,
                                            tag="py", name=f"phB{wi}")
                                for wi in range(len(widths))]
                        for kt in range(KT):
                            for phs, hbf in ((phsA, 0), (phsB, 1)):
                                if hbf == 0:
                                    lhsT = (w1a[:] if kt == 0 else
                                            w1b[:, kt * P:(kt + 1) * P])
                                else:
                                    lhsT = w1t1[:, kt * P:(kt + 1) * P]
                                tw0 = 0
                                for wi, tw in enumerate(widths):
                                    nc.tensor.matmul(
                                        phs[wi][:, :tw],
                                        lhsT,
                                        xs[kt][:, tw0:tw0 + tw],
                                        start=(kt == 0), stop=(kt == KT - 1))
                                    tw0 += tw
                        for phs, hbf in ((phsA, 0), (phsB, 1)):
                            bias = b1t[:, hbf:hbf + 1]
                            tw0 = 0
                            for wi, tw in enumerate(widths):
                                nc.scalar.activation(
                                    hs[hbf][:, tw0:tw0 + tw],
                                    phs[wi][:, :tw], gelu, bias=bias)
                                tw0 += tw
                        hb_lo = 2
                    for hb in range(hb_lo, HPB):
                        hid0 = b * HPB + hb
                        bias = b1t[:, g * 64 + hid0:g * 64 + hid0 + 1]
                        w1t = w1pool.tile([P, D_IN], bf16, tag="w1")
                        nc.sync.dma_start(w1t[:], w1p[g, hid0])
                        tw0 = 0
                        for tw in widths:
                            ph = phpool.tile([P, 512], mybir.dt.float32,
                                             tag="ph")
                            for kt in range(KT):
                                nc.tensor.matmul(
                                    ph[:, :tw],
                                    w1t[:, kt * P:(kt + 1) * P],
                                    xs[kt][:, tw0:tw0 + tw],
                                    start=(kt == 0), stop=(kt == KT - 1))
                            nc.scalar.activation(
                                hs[hb][:, tw0:tw0 + tw], ph[:, :tw],
                                gelu, bias=bias)
                            tw0 += tw
                    # ---- matmul 2: yT[dout, tok] += w2_blk.T-tiles @ h ----
                    # stationary = w2 [hid128, dout128] tiles, moving = h
                    # token chunks; tokens are exact (no ceil-to-128 waste)
                    for q in range(NDC):
                        w2t = w2pool.tile([P, HPB * NDC * P], bf16, tag="w2")
                        nc.sync.dma_start(w2t[:], w2p[g, b, q])
                        for dtl in range(NDC):
                            dt = q * NDC + dtl
                            ch0 = 0
                            for cw in widths:
                                py = pypool.tile([P, DOUTW], mybir.dt.float32,
                                                 tag="py")
                                for i in range(HPB):
                                    nc.tensor.matmul(
                                        py[:, :cw],
                                        w2t[:, (i * NDC + dtl) * P:
                                            (i * NDC + dtl + 1) * P],
                                        hs[i][:, ch0:ch0 + cw],
                                        start=(i == 0), stop=(i == HPB - 1))
                                dst = ys[dt][:, ch0:ch0 + cw]
                                if b == 0:
                                    nc.vector.tensor_copy(dst, py[:, :cw])
                                else:
                                    nc.vector.tensor_add(dst, dst, py[:, :cw])
                                    if b == NBLK - 1:
                                        nc.sync.dma_start(
                                            y[dt * P:(dt + 1) * P,
                                              g0 + ch0:g0 + ch0 + cw],
                                            dst)
                                ch0 += cw
                g0 += tg
    nc.compile()
    return nc


def _ensure_axon_hooks():
    """run_bass_kernel_spmd imports antenv.axon_hooks when tracing is
    requested (BASS_TRACE=1); provide an inert fallback if the optional
    module is absent so tracing degrades gracefully instead of crashing."""
    import importlib
    try:
        importlib.import_module("antenv.axon_hooks")
    except ImportError:
        import sys
        import types
        m = types.ModuleType("antenv.axon_hooks")
        m._hook = None
        m.set_axon_ntff_profile_hook = lambda h: setattr(m, "_hook", h)
        m.get_axon_ntff_profile_hook = lambda: m._hook
        sys.modules["antenv.axon_hooks"] = m


def _pack_w1(w1e):
    w1p = (w1e.reshape(D_HID // P, P, KT, P)
           .transpose(0, 3, 2, 1)
           .reshape(D_HID // P, P, D_IN))
    return np.ascontiguousarray(w1p)


def _pack_w2(w2e):
    w2p = (w2e.reshape(NDC, NDC, P, NBLK, HPB, P)     # [q, dtl, d, b, i, p]
           .transpose(3, 0, 5, 4, 1, 2)               # [b, q, p, i, dtl, d]
           .reshape(NBLK, NDC, P, HPB * NDC * P))
    return np.ascontiguousarray(w2p)


def kernel(x, gate_w, w1, b1, w2, b2):
    global LAST_EXEC_TIME_NS, LAST_RESULTS
    x = np.asarray(x, dtype=np.float32)
    gate_w = np.asarray(gate_w, dtype=np.float32)
    w1 = np.asarray(w1, dtype=np.float32)
    b1 = np.asarray(b1, dtype=np.float32)
    w2 = np.asarray(w2, dtype=np.float32)
    b2 = np.asarray(b2, dtype=np.float32)
    B = x.shape[0]

    # ---- host router (fp32, matches jax.lax.top_k tie-breaking) ----
    logits = x @ gate_w.T                                     # [B, E]
    order = np.argsort(-logits, axis=1, kind="stable")[:, :TOP_K]
    top_v = np.take_along_axis(logits, order, axis=1)
    mx = top_v.max(axis=1, keepdims=True)
    ex = np.exp(top_v - mx)
    coefs = ex / ex.sum(axis=1, keepdims=True)                # [B, 2]

    toks, cfs = [], []
    for e in range(NUM_EXPERTS):
        mask = order == e                                     # [B, 2]
        tok = np.nonzero(mask.any(axis=1))[0]
        first = mask[tok, 0]
        cf = np.where(first, coefs[tok, 0], coefs[tok, 1]).astype(np.float32)
        toks.append(tok)
        cfs.append(cf)

    groups, cores = _plan_segments([len(t) for t in toks])
    C = sum(groups)
    ngrp = len(groups)

    # pack each expert's weights once (experts can appear in 2-3 segments)
    w1packed = [_pack_w1(w1[e].astype(_BF16)) for e in range(NUM_EXPERTS)]
    w2packed = [_pack_w2(w2[e].astype(_BF16)) for e in range(NUM_EXPERTS)]
    b1cols = [np.ascontiguousarray(b1[e].reshape(D_HID // P, P).T)
              for e in range(NUM_EXPERTS)]

    # ---- per-core inputs: per-segment tokens + owned experts' weights ----
    in_maps = []
    for core_segs in cores:
        xg = np.zeros((C, D_IN), np.float32)
        b1c = np.zeros((P, ngrp * (D_HID // P)), np.float32)
        g0 = 0
        w1ps, w2ps = [], []
        for g, (e, s, l) in enumerate(core_segs):
            xg[g0:g0 + l] = x[toks[e][s:s + l]]
            w1ps.append(w1packed[e])
            w2ps.append(w2packed[e])
            b1c[:, g * 64:(g + 1) * 64] = b1cols[e]
            g0 += groups[g]
        in_maps.append({
            "xT": xg.T.astype(_BF16),
            "w1p": np.stack(w1ps),
            "w2p": np.stack(w2ps),
            "b1c": b1c,
        })

    nc = _nc_cache.get(groups)
    if nc is None:
        nc = _build_bass(groups)
        _nc_cache[groups] = nc

    _ensure_axon_hooks()
    from concourse.bass_utils import run_bass_kernel_spmd
    res = run_bass_kernel_spmd(nc, in_maps, core_ids=list(range(NUM_EXPERTS)))
    LAST_EXEC_TIME_NS = res.exec_time_ns
    LAST_RESULTS = res

    # ---- combine (unshard): weighted scatter-add; b2[e] folded in here ----
    out = np.zeros((B, D_OUT), np.float32)
    for ci, core_segs in enumerate(cores):
        yT = np.asarray(res.results[ci]["y"]).T                # [C, D_OUT]
        g0 = 0
        for g, (e, s, l) in enumerate(core_segs):
            if l:
                tok = toks[e][s:s + l]
                out[tok] += ((yT[g0:g0 + l] + b2[e][None, :])
                             * cfs[e][s:s + l, None])
            g0 += groups[g]
    return out


# revision 19
# speedup vs baseline: 1.0019x; 1.0019x over previous
"""MoE layer (8 experts, top-2) on 8 Trainium2 NeuronCores.

Strategy: expert parallelism with 2-segment load balancing. The router
(x @ gate_w.T -> top-2 -> softmax) is computed on host in fp32 (0.03% of
total FLOPs). Tokens are sharded BY EXPERT, but instead of padding every
core to the max expert load (2084 for this seed), each core runs TWO
fixed-length segments (L1, L2) with independent weight sets; expert token
lists are split across segments on different cores so the per-core
capacity drops to the optimal 2-segment covering (2068 = L1 1042 +
L2 1026 here, vs the 2048 ideal). Weight DMA traffic is unchanged (the
full per-expert weight set was already streamed once per token group).

Each core computes the dense expert MLP
    y = gelu(x @ w1[e].T + b1[e]) @ w2[e].T
in bf16 (fp32 PSUM accumulation). The combine (scatter-add weighted by
the top-2 softmax coefficients, with b2[e] folded in per expert) happens
on host as the unshard step.

Device kernel layout (per core, SPMD identical program):
  xT  [D_IN, C]  bf16   tokens, transposed (partition dim = contraction)
  w1p [2, 64, 128, 2048]   per-segment w1 tiles:
                        w1p[g, h0, p, kt*128+h] = w1[e_g][h0*128+h, kt*128+p]
  w2p [2, 8, 4, 128, 4096] per-segment w2 [hid128, dout128] tiles, grouped
                        by (hid block, dout quarter) so matmul-2 keeps w2
                        stationary and streams token columns
  b1c [128, 128]        b1 per hid-tile column, both segments packed
  y   [D_OUT, C] f32    expert output, transposed (excl. b2/routing coef)

The tensor engine runs at the bf16 issue-rate floor in steady state
(measured 162.5 ns per FD-384 matmul = N/2.4GHz + 2.5ns dispatch), so
the remaining overheads addressed here: HAM cold-clock ramp (warmup
matmuls on a zeroed tile during the DMA prologue), prologue DMA ordering
(first matmul waits only on a 32KB weight chunk + 128KB token chunk),
and capacity padding (segment covering above).
"""

import numpy as np
import ml_dtypes

TOP_K = 2
NUM_EXPERTS = 8
D_IN, D_HID, D_OUT = 2048, 8192, 2048

P = 128
KT = D_IN // P     # 16 contraction tiles
NBLK = 8           # hid blocks of 1024
HPB = 8            # hid 128-tiles per block
NDC = 4            # dout quarters; D_OUT = 4 * 512
DOUTW = 512
_BF16 = ml_dtypes.bfloat16

_nc_cache: dict[tuple, object] = {}

LAST_EXEC_TIME_NS = None
LAST_RESULTS = None


def _widths_for(tg: int) -> list[int]:
    """Split a group into the minimum number (ceil(tg/512)) of matmul
    moving widths, 128-aligned starts except the last, every chunk >= 64
    (the tensor engine has a ~60-cycle issue floor)."""
    n = -(-tg // 512)
    if n <= 1:
        return [tg]
    ws = [512] * (n - 2)
    rem = tg - 512 * (n - 2)
    if rem - 512 >= 64:
        ws += [512, rem - 512]
    else:
        ws += [384, rem - 384]
    assert all(w >= 64 for w in ws) and sum(ws) == tg
    return ws


def _plan_segments(counts):
    """Optimal 2-segment covering: per-core capacity C = L1 + L2 with the
    8 experts' token lists packed into 8 L1-segments + 8 L2-segments
    (each segment single-expert). Returns (L1, L2) and per-core
    [(expert, start, len), (expert, start, len)]."""
    counts = np.asarray(counts)
    order = np.argsort(-counts, kind="stable")
    cs = counts[order]
    best = None
    for n1 in range(0, 5):
        n3, n2 = n1, 8 - 2 * n1
        if n2 < 0:
            continue
        L1 = int(-(-cs[:n1].max() // 2)) if n1 else 0
        req_mid = int(cs[n1:n1 + n2].max()) if n2 else 0
        L2 = int(-(-cs[n1 + n2:].max() // 2)) if n3 else 0
        if n1 == 0:
            # all experts whole on one core: L1+L2 >= max count
            C = max(req_mid, 2 * 512)
            L1 = C - C // 2
        else:
            C = max(L1 + L2, req_mid, L1 + 512)
        if best is None or C < best[0]:
            best = (C, n1, n2, L1, L2)
    C, n1, n2, L1min, L2min = best
    # slide the split within [L1min, C - L2min] to minimize the total
    # matmul width-chunk count (2.5ns dispatch per chunk per kt pass)
    L1min, L2min = max(L1min, 512), max(L2min, 512)
    L1 = min(
        range(L1min, C - L2min + 1),
        key=lambda l1: (len(_widths_for(l1)) + len(_widths_for(C - l1)),
                        -min(_widths_for(l1) + _widths_for(C - l1))),
    )
    L2 = C - L1
    segs1, segs2 = [], []
    for i, e in enumerate(order):
        c = int(counts[e])
        if i < n1:                       # two L1 segments
            segs1.append((e, 0, min(L1, c)))
            segs1.append((e, L1, max(0, c - L1)))
        elif i < n1 + n2:                # L1 + L2
            segs1.append((e, 0, min(L1, c)))
            segs2.append((e, L1, max(0, c - L1)))
        else:                            # two L2 segments
            segs2.append((e, 0, min(L2, c)))
            segs2.append((e, L2, max(0, c - L2)))
    assert len(segs1) == len(segs2) == 8
    for e, s, l in segs1:
        assert l <= L1
    for e, s, l in segs2:
        assert l <= L2
    cores = [(segs1[i], segs2[i]) for i in range(8)]
    return (L1, L2), cores


def _build_bass(groups: tuple[int, ...]):
    from concourse import bacc
    import concourse.mybir as mybir
    import concourse.tile as tile

    bf16 = mybir.dt.bfloat16
    f32 = mybir.dt.float32
    C = sum(groups)
    tgmax = max(groups)
    ngrp = len(groups)

    nc = bacc.Bacc("TRN2", target_bir_lowering=False, debug=False,
                   num_devices=NUM_EXPERTS)
    # per-segment token arrays: a single [D_IN, C] matrix would make every
    # per-group column-slice DMA read partial rows (~50% HBM page
    # efficiency), which starves the chip-bandwidth-bound prologue
    xts = [nc.declare_dram_parameter(f"xT{g}", [D_IN, tg], bf16,
                                     isOutput=False)
           for g, tg in enumerate(groups)]
    w1p = nc.declare_dram_parameter("w1p", [ngrp, D_HID // P, P, D_IN], bf16,
                                    isOutput=False)
    w2p = nc.declare_dram_parameter("w2p", [ngrp, NBLK, NDC, P, HPB * NDC * P],
                                    bf16, isOutput=False)
    b1c = nc.declare_dram_parameter("b1c", [P, ngrp * (D_HID // P)], f32,
                                    isOutput=False)
    y = nc.declare_dram_parameter("y", [D_OUT, C], f32, isOutput=True)

    gelu = mybir.ActivationFunctionType.Gelu

    with tile.TileContext(nc) as tc:
        with (
            tc.tile_pool(name="consts", bufs=1) as cpool,
            tc.tile_pool(name="xpool", bufs=1) as xpool,
            tc.tile_pool(name="ypool", bufs=1) as ypool,
            tc.tile_pool(name="hpool", bufs=2) as hpool,
            tc.tile_pool(name="w1pool", bufs=3) as w1pool,
            tc.tile_pool(name="w2pool", bufs=3) as w2pool,
            tc.tile_pool(name="phpool", bufs=4, space="PSUM") as phpool,
            tc.tile_pool(name="pypool", bufs=4, space="PSUM") as pypool,
        ):
            # prologue: the first accumulation (kt 0) waits only on a 32KB
            # weight chunk and a 96KB token chunk. The startup burst
            # (w1 tile + 16 token tiles, ~4.8MB) is split across the two
            # hardware DGE queues (SP + Activation) so it keeps pace with
            # the first DMA-paced hid-tile pass; those matmuls double as
            # the HAM clock-gate warmup.
            w0 = _widths_for(groups[0])[0]
            xs0 = xpool.tile([P, tgmax], bf16, tag="x0", name="xs0")
            nc.sync.dma_start(xs0[:, :w0], xts[0][0:P, 0:w0])
            w1a = cpool.tile([P, P], bf16, tag="w1a")
            nc.scalar.dma_start(w1a[:], w1p[0, 0, :, :P])
            w1b = w1pool.tile([P, D_IN], bf16, tag="w1")
            nc.scalar.dma_start(w1b[:, P:], w1p[0, 0, :, P:])
            if w0 < groups[0]:
                nc.sync.dma_start(xs0[:, w0:groups[0]],
                                  xts[0][0:P, w0:groups[0]])
            b1t = cpool.tile([P, ngrp * (D_HID // P)], f32)

            g0 = 0
            for g, tg in enumerate(groups):
                widths = _widths_for(tg)
                xs = [xs0 if (g == 0 and kt == 0) else
                      xpool.tile([P, tgmax], bf16, tag=f"x{kt}",
                                 name=f"xs{kt}")
                      for kt in range(KT)]
                if g == 0:
                    nc.sync.dma_start(
                        xs[1][:, :tg], xts[0][P:2 * P, :tg])
                    nc.scalar.dma_start(b1t[:], b1c[:])
                    for kt in range(2, KT):
                        eng = nc.sync if kt % 2 == 0 else nc.scalar
                        eng.dma_start(
                            xs[kt][:, :tg],
                            xts[0][kt * P:(kt + 1) * P, :tg])
                else:
                    for kt in range(KT):
                        nc.sync.dma_start(
                            xs[kt][:, :tg],
                            xts[g][kt * P:(kt + 1) * P, :tg])
                ys = [ypool.tile([P, tgmax], f32, tag=f"y{t}", name=f"ys{t}")
                      for t in range(D_OUT // P)]
                for b in range(NBLK):
                    hs = [hpool.tile([P, tgmax], bf16, tag=f"h{i}",
                                     name=f"hs{i}")
                          for i in range(HPB)]
                    # ---- matmul 1: h[hid, tok] = w1 @ x, gelu ----
                    hb_lo = 0
                    if g == 0 and b == 0:
                        # The startup x/w burst (~5MB) is chip-HBM-bound
                        # (~13us with all 8 cores loading at once), so run
                        # the first TWO hid-tiles fused kt-major: ~14us of
                        # matmul paced against the arriving xs tiles keeps
                        # the PE gapless (doubles as HAM clock warmup).
                        # 6 concurrent PSUM accumulation groups: 3 from
                        # phpool + 3 borrowed from pypool (mm2 starts
                        # long after these drain).
                        w1t1 = w1pool.tile([P, D_IN], bf16, tag="w1",
                                           name="w1t1")
                        nc.scalar.dma_start(w1t1[:], w1p[0, 1])
                        phsA = [phpool.tile([P, 512], mybir.dt.float32,
                                            tag="ph", name=f"phA{wi}")
                                for wi in range(len(widths))]
                        phsB = [pypool.tile([P, DOUTW], mybir.dt.float32,
                                            tag="py", name=f"phB{wi}")
                                for wi in range(len(widths))]
                        for kt in range(KT):
                            for phs, hbf in ((phsA, 0), (phsB, 1)):
                                if hbf == 0:
                                    lhsT = (w1a[:] if kt == 0 else
                                            w1b[:, kt * P:(kt + 1) * P])
                                else:
                                    lhsT = w1t1[:, kt * P:(kt + 1) * P]
                                tw0 = 0
                                for wi, tw in enumerate(widths):
                                    nc.tensor.matmul(
                                        phs[wi][:, :tw],
                                        lhsT,
                                        xs[kt][:, tw0:tw0 + tw],
                                        start=(kt == 0), stop=(kt == KT - 1))
                                    tw0 += tw
                        for phs, hbf in ((phsA, 0), (phsB, 1)):
                            bias = b1t[:, hbf:hbf + 1]
                            tw0 = 0
                            for wi, tw in enumerate(widths):
                                nc.scalar.activation(
                                    hs[hbf][:, tw0:tw0 + tw],
                                    phs[wi][:, :tw], gelu, bias=bias)
                                tw0 += tw
                        hb_lo = 2
                    for hb in range(hb_lo, HPB):
                        hid0 = b * HPB + hb
                        bias = b1t[:, g * 64 + hid0:g * 64 + hid0 + 1]
                        w1t = w1pool.tile([P, D_IN], bf16, tag="w1")
                        nc.sync.dma_start(w1t[:], w1p[g, hid0])
                        tw0 = 0
                        for tw in widths:
                            ph = phpool.tile([P, 512], mybir.dt.float32,
                                             tag="ph")
                            for kt in range(KT):
                                nc.tensor.matmul(
                                    ph[:, :tw],
                                    w1t[:, kt * P:(kt + 1) * P],
                                    xs[kt][:, tw0:tw0 + tw],
                                    start=(kt == 0), stop=(kt == KT - 1))
                            nc.scalar.activation(
                                hs[hb][:, tw0:tw0 + tw], ph[:, :tw],
                                gelu, bias=bias)
                            tw0 += tw
                    # ---- matmul 2: yT[dout, tok] += w2_blk.T-tiles @ h ----
                    # stationary = w2 [hid128, dout128] tiles, moving = h
                    # token chunks; tokens are exact (no ceil-to-128 waste)
                    for q in range(NDC):
                        w2t = w2pool.tile([P, HPB * NDC * P], bf16, tag="w2")
                        nc.sync.dma_start(w2t[:], w2p[g, b, q])
                        for dtl in range(NDC):
                            dt = q * NDC + dtl
                            ch0 = 0
                            for cw in widths:
                                py = pypool.tile([P, DOUTW], mybir.dt.float32,
                                                 tag="py")
                                for i in range(HPB):
                                    nc.tensor.matmul(
                                        py[:, :cw],
                                        w2t[:, (i * NDC + dtl) * P:
                                            (i * NDC + dtl + 1) * P],
                                        hs[i][:, ch0:ch0 + cw],
                                        start=(i == 0), stop=(i == HPB - 1))
                                dst = ys[dt][:, ch0:ch0 + cw]
                                if b == 0:
                                    nc.vector.tensor_copy(dst, py[:, :cw])
                                else:
                                    nc.vector.tensor_add(dst, dst, py[:, :cw])
                                    if b == NBLK - 1:
                                        nc.sync.dma_start(
                                            y[dt * P:(dt + 1) * P,
                                              g0 + ch0:g0 + ch0 + cw],
                                            dst)
                                ch0 += cw
                g0 += tg
    nc.compile()
    return nc


def _ensure_axon_hooks():
    """run_bass_kernel_spmd imports antenv.axon_hooks when tracing is
    requested (BASS_TRACE=1); provide an inert fallback if the optional
    module is absent so tracing degrades gracefully instead of crashing."""
    import importlib
    try:
        importlib.import_module("antenv.axon_hooks")
    except ImportError:
        import sys
        import types
        m = types.ModuleType("antenv.axon_hooks")
        m._hook = None
        m.set_axon_ntff_profile_hook = lambda h: setattr(m, "_hook", h)
        m.get_axon_ntff_profile_hook = lambda: m._hook
        sys.modules["antenv.axon_hooks"] = m


def _pack_w1(w1e):
    w1p = (w1e.reshape(D_HID // P, P, KT, P)
           .transpose(0, 3, 2, 1)
           .reshape(D_HID // P, P, D_IN))
    return np.ascontiguousarray(w1p)


def _pack_w2(w2e):
    w2p = (w2e.reshape(NDC, NDC, P, NBLK, HPB, P)     # [q, dtl, d, b, i, p]
           .transpose(3, 0, 5, 4, 1, 2)               # [b, q, p, i, dtl, d]
           .reshape(NBLK, NDC, P, HPB * NDC * P))
    return np.ascontiguousarray(w2p)


def kernel(x, gate_w, w1, b1, w2, b2):
    global LAST_EXEC_TIME_NS, LAST_RESULTS
    x = np.asarray(x, dtype=np.float32)
    gate_w = np.asarray(gate_w, dtype=np.float32)
    w1 = np.asarray(w1, dtype=np.float32)
    b1 = np.asarray(b1, dtype=np.float32)
    w2 = np.asarray(w2, dtype=np.float32)
    b2 = np.asarray(b2, dtype=np.float32)
    B = x.shape[0]

    # ---- host router (fp32, matches jax.lax.top_k tie-breaking) ----
    logits = x @ gate_w.T                                     # [B, E]
    order = np.argsort(-logits, axis=1, kind="stable")[:, :TOP_K]
    top_v = np.take_along_axis(logits, order, axis=1)
    mx = top_v.max(axis=1, keepdims=True)
    ex = np.exp(top_v - mx)
    coefs = ex / ex.sum(axis=1, keepdims=True)                # [B, 2]

    toks, cfs = [], []
    for e in range(NUM_EXPERTS):
        mask = order == e                                     # [B, 2]
        tok = np.nonzero(mask.any(axis=1))[0]
        first = mask[tok, 0]
        cf = np.where(first, coefs[tok, 0], coefs[tok, 1]).astype(np.float32)
        toks.append(tok)
        cfs.append(cf)

    groups, cores = _plan_segments([len(t) for t in toks])
    C = sum(groups)
    ngrp = len(groups)

    # pack each expert's weights once (experts can appear in 2-3 segments)
    w1packed = [_pack_w1(w1[e].astype(_BF16)) for e in range(NUM_EXPERTS)]
    w2packed = [_pack_w2(w2[e].astype(_BF16)) for e in range(NUM_EXPERTS)]
    b1cols = [np.ascontiguousarray(b1[e].reshape(D_HID // P, P).T)
              for e in range(NUM_EXPERTS)]

    # ---- per-core inputs: per-segment tokens + owned experts' weights ----
    in_maps = []
    for core_segs in cores:
        b1c = np.zeros((P, ngrp * (D_HID // P)), np.float32)
        im = {"b1c": b1c}
        w1ps, w2ps = [], []
        for g, (e, s, l) in enumerate(core_segs):
            xg = np.zeros((groups[g], D_IN), np.float32)
            xg[:l] = x[toks[e][s:s + l]]
            im[f"xT{g}"] = xg.T.astype(_BF16)
            w1ps.append(w1packed[e])
            w2ps.append(w2packed[e])
            b1c[:, g * 64:(g + 1) * 64] = b1cols[e]
        im["w1p"] = np.stack(w1ps)
        im["w2p"] = np.stack(w2ps)
        in_maps.append(im)

    nc = _nc_cache.get(groups)
    if nc is None:
        nc = _build_bass(groups)
        _nc_cache[groups] = nc

    _ensure_axon_hooks()
    from concourse.bass_utils import run_bass_kernel_spmd
    res = run_bass_kernel_spmd(nc, in_maps, core_ids=list(range(NUM_EXPERTS)))
    LAST_EXEC_TIME_NS = res.exec_time_ns
    LAST_RESULTS = res

    # ---- combine (unshard): weighted scatter-add; b2[e] folded in here ----
    out = np.zeros((B, D_OUT), np.float32)
    for ci, core_segs in enumerate(cores):
        yT = np.asarray(res.results[ci]["y"]).T                # [C, D_OUT]
        g0 = 0
        for g, (e, s, l) in enumerate(core_segs):
            if l:
                tok = toks[e][s:s + l]
                out[tok] += ((yT[g0:g0 + l] + b2[e][None, :])
                             * cfs[e][s:s + l, None])
            g0 += groups[g]
    return out
